# revision 43
# baseline (speedup 1.0000x reference)
"""Trainium2 Bass kernel for nn_Attention (B,H,W,n,dim)=(2,64,64,8,512), 8 heads x 64.

Per core: 1024 pixels = 8192 rows of (pixel,token), dim 512.
Row-tiles of 128 rows (16 pixels), 64 tiles, grouped 4 tiles = 512 rows.

Pipeline (software-pipelined, skew 4 tiles):
  - host pre-transposes x -> xT [512, 8192] (no PE transposes needed).
  - qkT per GROUP: stationary = w block [128,128], moving = xT chunk
    [128, 512 rows] -> q,k feature blocks [128 feat, 512 rows]; 8 LDW/tile.
  - v natural per tile: stationary = xT chunk [128,128], moving = w_v
    [128, 512].
  - scores per head: lhsT=qT[64,128], rhs=kT[64,128] -> S [128,128] psum
    (2 psum tiles of 4 heads each).
  - softmax batched on [128, 8, 128] broadcast APs:
    mask-mult (Pool, from psum) -> exp (Act) -> zero via bmask (DVE) ->
    windowed reduce (DVE) -> reciprocal bf16 (DVE) -> normalize via
    broadcast recip (DVE) -> 32-block transpose (DVE) -> attnT.
  - AV directly transposed: lhsT = v_h [128 rows, 64], rhs = attnT_h
    [128, 128] -> avT [128 inner, 128 rows] assembled per-chunk in psum.
  - out-proj: lhsT = avT chunk, rhs = w_out chunk [128, 512] -> out
    [128 rows, 512] f32 -> evac (Pool) -> DMA.
"""

import os
import sys

sys.path.insert(0, "/opt/trn_rl_repo")

import numpy as np
import ml_dtypes

import concourse.bass as bass
import concourse.bacc as bacc
import concourse.mybir as mybir
import concourse.tile as tile
from concourse.bass_utils import run_bass_kernel_spmd

HEADS = 8
DIM_HEAD = 64
SCALE = DIM_HEAD ** (-0.5)
B, H, W, NTOK, DIM = 2, 64, 64, 8, 512
INNER = HEADS * DIM_HEAD  # 512
N_CORES = 8
PIX_TOTAL = B * H * W          # 8192
PIX_CORE = PIX_TOTAL // N_CORES  # 1024
ROWS = PIX_CORE * NTOK         # 8192 rows per core
RT = 128                       # rows per tile (16 pixels)
NT = ROWS // RT                # 64 tiles
GSZ = 4                        # tiles per group (512 rows)
NT_BUILD = int(os.environ.get("KERNEL_NT", NT))  # reduced build for sim tests
# debug stage gating: 1=qkT+v, 2=+scores, 3=+mask, 4=+softmax, 5=full (default)
STAGE = int(os.environ.get("KERNEL_STAGE", 5))

BF16 = mybir.dt.bfloat16
F32 = mybir.dt.float32

_cache = {}


def build_nc(nt=NT_BUILD):
    assert nt % GSZ == 0
    nc = bacc.Bacc()
    xT_d = nc.declare_dram_parameter("xT", [DIM, ROWS], BF16, isOutput=False)
    wqkv_d = nc.declare_dram_parameter("wqkv", [DIM, 3 * INNER], BF16, isOutput=False)
    wout_d = nc.declare_dram_parameter("wout", [INNER, DIM], BF16, isOutput=False)
    maskp_d = nc.declare_dram_parameter("maskp", [ROWS, 128], BF16, isOutput=False)
    bmask_d = nc.declare_dram_parameter("bmask", [128, 128], BF16, isOutput=False)
    out_d = nc.declare_dram_parameter("out", [ROWS, DIM], BF16, isOutput=True)

    MULT = mybir.AluOpType.mult
    ADD = mybir.AluOpType.add
    AXX = mybir.AxisListType.X
    EXP = mybir.ActivationFunctionType.Exp

    with tile.TileContext(nc) as tc:
        with (
            tc.tile_pool(name="const", bufs=1) as constp,
            tc.tile_pool(name="qkt", bufs=2) as qktp,
            tc.tile_pool(name="vsb", bufs=8) as vsbp,
            tc.tile_pool(name="sfx", bufs=3) as sfxp,
            tc.tile_pool(name="att", bufs=8) as attp,
            tc.tile_pool(name="avs", bufs=2) as avsp,
            tc.tile_pool(name="osb", bufs=3) as osbp,
            tc.tile_pool(name="ps_qk", bufs=2, space="PSUM") as ps_qk,
            tc.tile_pool(name="ps_v", bufs=1, space="PSUM") as ps_v,
            tc.tile_pool(name="ps_sc", bufs=1, space="PSUM") as ps_sc,
            tc.tile_pool(name="ps_av", bufs=1, space="PSUM") as ps_av,
            tc.tile_pool(name="ps_o", bufs=1, space="PSUM") as ps_o,
        ):
            # resident constants
            wq_sb = constp.tile([128, 4 * 1536], BF16, name="wq_sb")
            wo_sb = constp.tile([128, 4 * 512], BF16, name="wo_sb")
            bmask = constp.tile([128, 128], BF16, name="bmask_sb")
            for c in range(4):
                nc.sync.dma_start(out=wq_sb[:, c * 1536:(c + 1) * 1536],
                                  in_=wqkv_d[c * 128:(c + 1) * 128, :])
                nc.sync.dma_start(out=wo_sb[:, c * 512:(c + 1) * 512],
                                  in_=wout_d[c * 128:(c + 1) * 128, :])
            nc.sync.dma_start(out=bmask[:], in_=bmask_d[:])
            # full pre-transposed x shard and mask' resident in SBUF
            xfT = constp.tile([128, 4 * ROWS], BF16, name="xfT")
            for c in range(4):
                nc.sync.dma_start(out=xfT[:, c * ROWS:(c + 1) * ROWS],
                                  in_=xT_d[c * 128:(c + 1) * 128, :])
            mfull = constp.tile([128, nt * 128], BF16, name="mfull")
            for tt in range(nt):
                nc.sync.dma_start(out=mfull[:, tt * 128:(tt + 1) * 128],
                                  in_=maskp_d[tt * RT:(tt + 1) * RT, :])

            bmask_b = bmask[:].rearrange("p (o c) -> p o c", o=1)\
                              .to_broadcast([128, 8, 128])

            saved = {}  # t -> (v_sb, attnT)
            # Head processing order: even heads first (lhsT partition base 0,
            # into scA/pA), then odd heads (base 64, scB/pB). Consecutive
            # matmuls into one psum bank keep a CONSTANT stationary partition
            # base — alternating bases into a shared bank wedges the PE
            # (NRT_EXEC_UNIT_UNRECOVERABLE, found by HW probe bisect).
            HORD = [0, 2, 4, 6, 1, 3, 5, 7]

            def emit_back(t):
                v_sb, attnT = saved.pop(t)
                # ---- AV, directly transposed: per head out [64,128].
                # Even heads -> pA partitions 0-63, odd -> pB, at cols
                # 128*(h//2); avt_sb[64*(h%2)+d, 128*(h//2)+r] = avT chunk.
                pA = ps_av.tile([64, 512], F32, tag="avtA")
                pB = ps_av.tile([64, 512], F32, tag="avtB")
                for j, h in enumerate(HORD):
                    co = 128 * (h // 2)
                    dstp = pA if h % 2 == 0 else pB
                    nc.tensor.matmul(
                        dstp[:, co:co + 128],
                        lhsT=v_sb[:, h * 64:(h + 1) * 64],
                        rhs=attnT[:, j * 128:(j + 1) * 128],
                        start=(j % 4 == 0), stop=(j % 4 == 3),
                        skip_group_check=True,
                    )
                return (pA, pB)

            def emit_back2(t, avt_ps):
                pA, pB = avt_ps
                avt_sb = avsp.tile([128, 512], BF16, tag="avt_sb")
                nc.vector.tensor_copy(out=avt_sb[0:64, :], in_=pA[:])
                nc.scalar.copy(out=avt_sb[64:128, :], in_=pB[:])
                o_ps = ps_o.tile([128, 512], F32, tag="o")
                for c in range(4):
                    nc.tensor.matmul(
                        o_ps[:],
                        lhsT=avt_sb[:, c * 128:(c + 1) * 128],
                        rhs=wo_sb[:, c * 512:(c + 1) * 512],
                        start=(c == 0), stop=(c == 3),
                    )
                o_sb = osbp.tile([128, 512], BF16, tag="o_sb")
                nc.scalar.copy(out=o_sb[:], in_=o_ps[:])
                nc.scalar.dma_start(out=out_d[t * RT:(t + 1) * RT, :], in_=o_sb[:])

            def emit_group_qkT(g):
                # ---- qkT for the whole group: 8 feature blocks
                # [128 feat, 512 rows]; layout qk_g[:, fb*512 + s*128 + r]
                qk_g = qktp.tile([128, 8 * 512], BF16, tag="qk_g")
                for fb in range(8):
                    wcol = fb * 128 if fb < 4 else 512 + (fb - 4) * 128
                    qk_ps = ps_qk.tile([128, 512], F32, tag="qkfb")
                    for c in range(4):
                        nc.tensor.matmul(
                            qk_ps[:],
                            lhsT=wq_sb[:, c * 1536 + wcol: c * 1536 + wcol + 128],
                            rhs=xfT[:, c * ROWS + g * 512: c * ROWS + (g + 1) * 512],
                            start=(c == 0), stop=(c == 3),
                        )
                    nc.scalar.copy(out=qk_g[:, fb * 512:(fb + 1) * 512], in_=qk_ps[:])
                return qk_g

            def emit_soft(t):
                """exp + softmax tail for tile t (sm(t) was made last iter)."""
                v_sb, sm = saved[t]
                ex = sfxp.tile([128, 1024], BF16, tag="ex")
                nc.scalar.activation(ex[:], sm[:], EXP)
                z = sfxp.tile([128, 1024], BF16, tag="z")
                nc.vector.tensor_tensor(
                    out=z[:].rearrange("p (h c) -> p h c", h=8),
                    in0=ex[:].rearrange("p (h c) -> p h c", h=8),
                    in1=bmask_b, op=MULT)
                sums = sfxp.tile([128, 8], BF16, tag="sums")
                with nc.allow_low_precision(reason="bf16 softmax denom, tol 2e-2"):
                    nc.vector.tensor_reduce(
                        out=sums[:], in_=z[:].rearrange("p (h c) -> p h c", h=8),
                        axis=AXX, op=ADD)
                rec32 = sfxp.tile([128, 8], F32, tag="rec32")
                nc.vector.reciprocal(rec32[:], sums[:])
                rec = sfxp.tile([128, 8], BF16, tag="rec")
                nc.vector.tensor_copy(out=rec[:], in_=rec32[:])
                attnb = attp.tile([128, 1024], BF16, tag="attnb")
                nc.vector.tensor_tensor(
                    out=attnb[:].rearrange("p (h c) -> p h c", h=8),
                    in0=z[:].rearrange("p (h c) -> p h c", h=8),
                    in1=rec[:, :].to_broadcast([128, 8, 128]), op=MULT)
                attnT = attp.tile([128, 1024], BF16, tag="attnT")
                nc.vector.transpose(attnT[:], attnb[:])
                saved[t] = (v_sb, attnT)

            ngroups = nt // GSZ
            qk_g_next = None
            for g in range(ngroups):
                qk_g = emit_group_qkT(g) if g == 0 else qk_g_next
                for s in range(GSZ):
                    t = g * GSZ + s
                    tb = t - GSZ  # back-stage tile (softmax long done)
                    if s == GSZ - 1 and g + 1 < ngroups:
                        # emit next group's qkT one tile early: PE fills the
                        # Act-evac wait with this tile's work, and the evacs
                        # queue ahead of this tile's exp on Act
                        qk_g_next = emit_group_qkT(g + 1)
                    # ---- back stage part 1: AV matmuls
                    avt_ps = emit_back(tb) if (tb >= 0 and STAGE >= 5) else None
                    # ---- v natural [128 rows, 512]
                    v_ps = ps_v.tile([128, 512], F32, tag="v_ps")
                    for c in range(4):
                        nc.tensor.matmul(
                            v_ps[:],
                            lhsT=xfT[:, c * ROWS + t * 128: c * ROWS + (t + 1) * 128],
                            rhs=wq_sb[:, c * 1536 + 1024: c * 1536 + 1536],
                            start=(c == 0), stop=(c == 3),
                        )
                    # ---- back stage part 2: avT evac, out-proj, dma
                    if tb >= 0 and STAGE >= 5:
                        emit_back2(tb, avt_ps)
                    v_sb = vsbp.tile([128, 512], BF16, tag="v_sb")
                    nc.scalar.copy(out=v_sb[:], in_=v_ps[:])
                    if STAGE < 5:
                        nc.scalar.dma_start(out=out_d[t * RT:(t + 1) * RT, :],
                                            in_=v_sb[:])
                    if STAGE < 2:
                        continue

                    # ---- scores: even heads -> scA, odd -> scB
                    scA = ps_sc.tile([128, 512], F32, tag="scA")
                    scB = ps_sc.tile([128, 512], F32, tag="scB")
                    for j, h in enumerate(HORD):
                        pb = 64 * (h % 2)
                        qof = (h // 2) * 512 + s * 128
                        kof = (4 + h // 2) * 512 + s * 128
                        dst = scA if j < 4 else scB
                        nc.tensor.matmul(
                            dst[:, (j % 4) * 128:(j % 4) * 128 + 128],
                            lhsT=qk_g[pb:pb + 64, qof:qof + 128],
                            rhs=qk_g[pb:pb + 64, kof:kof + 128],
                            start=True, stop=True,
                        )
                    if STAGE < 3:
                        # still consume scA/scB so psum banks recycle
                        dbg = sfxp.tile([128, 1024], BF16, tag="sm")
                        nc.vector.tensor_copy(out=dbg[:, :512], in_=scA[:])
                        nc.vector.tensor_copy(out=dbg[:, 512:], in_=scB[:])
                        continue
                    # ---- mask-mult straight out of psum
                    mkp_b = mfull[:, t * 128:(t + 1) * 128]\
                        .rearrange("p (o c) -> p o c", o=1)\
                        .to_broadcast([128, 4, 128])
                    sm = sfxp.tile([128, 1024], BF16, tag="sm")
                    nc.vector.tensor_tensor(
                        out=sm[:, :512].rearrange("p (h c) -> p h c", h=4),
                        in0=scA[:].rearrange("p (h c) -> p h c", h=4),
                        in1=mkp_b, op=MULT)
                    nc.vector.tensor_tensor(
                        out=sm[:, 512:].rearrange("p (h c) -> p h c", h=4),
                        in0=scB[:].rearrange("p (h c) -> p h c", h=4),
                        in1=mkp_b, op=MULT)
                    saved[t] = (v_sb, sm)
                    if STAGE >= 4:
                        emit_soft(t)

            # drain last group's back stages
            if STAGE >= 5:
                for t in range(nt - GSZ, nt):
                    avt_ps = emit_back(t)
                    emit_back2(t, avt_ps)
    return nc


def host_inputs(x, mask, w_qkv, w_out):
    """Build per-core input maps (host-side layout/dtype prep only)."""
    bf = ml_dtypes.bfloat16
    x_rows = np.ascontiguousarray(x.reshape(PIX_TOTAL * NTOK, DIM)).astype(bf)
    wq = np.ascontiguousarray(w_qkv).astype(bf)
    wo = np.ascontiguousarray(w_out).astype(bf)

    # block indicator bmask[8*px + i, 8*px2 + j] = (px == px2), px over 16
    bm = np.zeros((16, 8, 16, 8), np.float32)
    for p in range(16):
        bm[p, :, p, :] = 1.0
    bmask = bm.reshape(128, 128).astype(bf)

    # mask' per row (px,i): cols (px'', j) = (px''==px%16) ? mask[px,i,j]*SCALE : 0
    m = mask.reshape(PIX_TOTAL, NTOK, NTOK).astype(np.float32)
    in_maps = []
    for cidx in range(N_CORES):
        xc = x_rows[cidx * ROWS:(cidx + 1) * ROWS]  # [8192, 512]
        xT = np.ascontiguousarray(xc.T)             # [512, 8192]
        mc = m[cidx * PIX_CORE:(cidx + 1) * PIX_CORE]  # [1024, 8, 8]
        mp = np.zeros((PIX_CORE, NTOK, 16, NTOK), np.float32)
        pl = np.arange(PIX_CORE) % 16
        mp[np.arange(PIX_CORE), :, pl, :] = mc * SCALE
        maskp = mp.reshape(ROWS, 128).astype(bf)
        in_maps.append({
            "xT": xT,
            "wqkv": wq,
            "wout": wo,
            "maskp": maskp,
            "bmask": bmask,
        })
    return in_maps


def kernel(x, mask, w_qkv, w_out, b_out):
    if "nc" not in _cache:
        nc0 = build_nc(NT)
        nc0.finalize()
        _cache["nc"] = nc0
    nc = _cache["nc"]
    in_maps = host_inputs(x, mask, w_qkv, w_out)
    trace = bool(os.environ.get("KERNEL_TRACE"))
    res = run_bass_kernel_spmd(nc, in_maps, list(range(N_CORES)), trace=trace)
    _cache["last_res"] = res
    outs = [np.asarray(r["out"]).astype(np.float32) for r in res.results]
    full = np.concatenate(outs, axis=0)  # [65536, 512]
    out = full.reshape(B, H, W, NTOK, DIM) + np.asarray(b_out, dtype=np.float32)
    return out.astype(np.float32)


# revision 46
# speedup vs baseline: 1.2551x; 1.2551x over previous
"""Trainium2 Bass kernel for nn_Attention (B,H,W,n,dim)=(2,64,64,8,512), 8 heads x 64.

Per core: 1024 pixels = 8192 rows of (pixel,token), dim 512.
Row-tiles of 128 rows (16 pixels), 64 tiles, grouped 4 tiles = 512 rows.

Pipeline (software-pipelined, skew 4 tiles):
  - host pre-transposes x -> xT [512, 8192] (no PE transposes needed).
  - qkT per GROUP: stationary = w block [128,128], moving = xT chunk
    [128, 512 rows] -> q,k feature blocks [128 feat, 512 rows]; 8 LDW/tile.
  - v natural per tile: stationary = xT chunk [128,128], moving = w_v
    [128, 512].
  - scores per head: lhsT=qT[64,128], rhs=kT[64,128] -> S [128,128] psum
    (2 psum tiles of 4 heads each).
  - softmax batched on [128, 8, 128] broadcast APs:
    mask-mult (Pool, from psum) -> exp (Act) -> zero via bmask (DVE) ->
    windowed reduce (DVE) -> reciprocal bf16 (DVE) -> normalize via
    broadcast recip (DVE) -> 32-block transpose (DVE) -> attnT.
  - AV directly transposed: lhsT = v_h [128 rows, 64], rhs = attnT_h
    [128, 128] -> avT [128 inner, 128 rows] assembled per-chunk in psum.
  - out-proj: lhsT = avT chunk, rhs = w_out chunk [128, 512] -> out
    [128 rows, 512] f32 -> evac (Pool) -> DMA.
"""

import os
import sys

sys.path.insert(0, "/opt/trn_rl_repo")

import numpy as np
import ml_dtypes

import concourse.bass as bass
import concourse.bacc as bacc
import concourse.mybir as mybir
import concourse.tile as tile
from concourse.bass_utils import run_bass_kernel_spmd

HEADS = 8
DIM_HEAD = 64
SCALE = DIM_HEAD ** (-0.5)
B, H, W, NTOK, DIM = 2, 64, 64, 8, 512
INNER = HEADS * DIM_HEAD  # 512
N_CORES = 8
PIX_TOTAL = B * H * W          # 8192
PIX_CORE = PIX_TOTAL // N_CORES  # 1024
ROWS = PIX_CORE * NTOK         # 8192 rows per core
RT = 128                       # rows per tile (16 pixels)
NT = ROWS // RT                # 64 tiles
GSZ = 4                        # tiles per group (512 rows)
NT_BUILD = int(os.environ.get("KERNEL_NT", NT))  # reduced build for sim tests
# debug stage gating: 1=qkT+v, 2=+scores, 3=+mask, 4=+softmax, 5=full (default)
STAGE = int(os.environ.get("KERNEL_STAGE", 5))

BF16 = mybir.dt.bfloat16
F32 = mybir.dt.float32

_cache = {}


def build_nc(nt=NT_BUILD):
    assert nt % GSZ == 0
    nc = bacc.Bacc()
    xT_d = nc.declare_dram_parameter("xT", [DIM, ROWS], BF16, isOutput=False)
    wqkv_d = nc.declare_dram_parameter("wqkv", [DIM, 3 * INNER], BF16, isOutput=False)
    wout_d = nc.declare_dram_parameter("wout", [INNER, DIM], BF16, isOutput=False)
    maskp_d = nc.declare_dram_parameter("maskp", [ROWS, 128], BF16, isOutput=False)
    bmask_d = nc.declare_dram_parameter("bmask", [128, 128], BF16, isOutput=False)
    out_d = nc.declare_dram_parameter("out", [ROWS, DIM], BF16, isOutput=True)

    MULT = mybir.AluOpType.mult
    ADD = mybir.AluOpType.add
    AXX = mybir.AxisListType.X
    EXP = mybir.ActivationFunctionType.Exp

    with tile.TileContext(nc) as tc:
        with (
            tc.tile_pool(name="const", bufs=1) as constp,
            tc.tile_pool(name="qkt", bufs=2) as qktp,
            tc.tile_pool(name="vsb", bufs=8) as vsbp,
            tc.tile_pool(name="sfx", bufs=3) as sfxp,
            tc.tile_pool(name="att", bufs=8) as attp,
            tc.tile_pool(name="avs", bufs=2) as avsp,
            tc.tile_pool(name="osb", bufs=3) as osbp,
            tc.tile_pool(name="ps_qk", bufs=2, space="PSUM") as ps_qk,
            tc.tile_pool(name="ps_v", bufs=1, space="PSUM") as ps_v,
            tc.tile_pool(name="ps_sc", bufs=1, space="PSUM") as ps_sc,
            tc.tile_pool(name="ps_av", bufs=1, space="PSUM") as ps_av,
            tc.tile_pool(name="ps_o", bufs=1, space="PSUM") as ps_o,
        ):
            # resident constants
            wq_sb = constp.tile([128, 4 * 1536], BF16, name="wq_sb")
            wo_sb = constp.tile([128, 4 * 512], BF16, name="wo_sb")
            bmask = constp.tile([128, 128], BF16, name="bmask_sb")
            for c in range(4):
                nc.sync.dma_start(out=wq_sb[:, c * 1536:(c + 1) * 1536],
                                  in_=wqkv_d[c * 128:(c + 1) * 128, :])
                nc.sync.dma_start(out=wo_sb[:, c * 512:(c + 1) * 512],
                                  in_=wout_d[c * 128:(c + 1) * 128, :])
            nc.sync.dma_start(out=bmask[:], in_=bmask_d[:])
            # full pre-transposed x shard and mask' resident in SBUF
            xfT = constp.tile([128, 4 * ROWS], BF16, name="xfT")
            for c in range(4):
                nc.sync.dma_start(out=xfT[:, c * ROWS:(c + 1) * ROWS],
                                  in_=xT_d[c * 128:(c + 1) * 128, :])
            mfull = constp.tile([128, nt * 128], BF16, name="mfull")
            for tt in range(nt):
                nc.sync.dma_start(out=mfull[:, tt * 128:(tt + 1) * 128],
                                  in_=maskp_d[tt * RT:(tt + 1) * RT, :])

            bmask_b = bmask[:].rearrange("p (o c) -> p o c", o=1)\
                              .to_broadcast([128, 8, 128])

            saved = {}  # t -> (v_sb, attnT)
            # Head processing order: even heads first (lhsT partition base 0,
            # into scA/pA), then odd heads (base 64, scB/pB). Consecutive
            # matmuls into one psum bank keep a CONSTANT stationary partition
            # base — alternating bases into a shared bank wedges the PE
            # (NRT_EXEC_UNIT_UNRECOVERABLE, found by HW probe bisect).
            HORD = [0, 2, 4, 6, 1, 3, 5, 7]

            def emit_back(t):
                v_sb, attnT = saved.pop(t)
                # ---- AV, directly transposed: per head out [64,128].
                # Even heads -> pA partitions 0-63, odd -> pB, at cols
                # 128*(h//2); avt_sb[64*(h%2)+d, 128*(h//2)+r] = avT chunk.
                pA = ps_av.tile([64, 512], F32, tag="avtA")
                pB = ps_av.tile([64, 512], F32, tag="avtB")
                for j, h in enumerate(HORD):
                    co = 128 * (h // 2)
                    dstp = pA if h % 2 == 0 else pB
                    nc.tensor.matmul(
                        dstp[:, co:co + 128],
                        lhsT=v_sb[:, h * 64:(h + 1) * 64],
                        rhs=attnT[:, j * 128:(j + 1) * 128],
                        start=(j % 4 == 0), stop=(j % 4 == 3),
                        skip_group_check=True,
                    )
                return (pA, pB)

            def emit_back2(t, avt_ps):
                pA, pB = avt_ps
                avt_sb = avsp.tile([128, 512], BF16, tag="avt_sb")
                nc.scalar.copy(out=avt_sb[0:64, :], in_=pA[:])
                nc.scalar.copy(out=avt_sb[64:128, :], in_=pB[:])
                o_ps = ps_o.tile([128, 512], F32, tag="o")
                for c in range(4):
                    nc.tensor.matmul(
                        o_ps[:],
                        lhsT=avt_sb[:, c * 128:(c + 1) * 128],
                        rhs=wo_sb[:, c * 512:(c + 1) * 512],
                        start=(c == 0), stop=(c == 3),
                    )
                o_sb = osbp.tile([128, 512], BF16, tag="o_sb")
                nc.scalar.copy(out=o_sb[:], in_=o_ps[:])
                nc.scalar.dma_start(out=out_d[t * RT:(t + 1) * RT, :], in_=o_sb[:])

            def emit_group_qkT(g):
                # ---- qkT for the whole group: 8 feature blocks
                # [128 feat, 512 rows]; layout qk_g[:, fb*512 + s*128 + r]
                qk_g = qktp.tile([128, 8 * 512], BF16, tag="qk_g")
                for fb in range(8):
                    wcol = fb * 128 if fb < 4 else 512 + (fb - 4) * 128
                    qk_ps = ps_qk.tile([128, 512], F32, tag="qkfb")
                    for c in range(4):
                        nc.tensor.matmul(
                            qk_ps[:],
                            lhsT=wq_sb[:, c * 1536 + wcol: c * 1536 + wcol + 128],
                            rhs=xfT[:, c * ROWS + g * 512: c * ROWS + (g + 1) * 512],
                            start=(c == 0), stop=(c == 3),
                        )
                    nc.scalar.copy(out=qk_g[:, fb * 512:(fb + 1) * 512], in_=qk_ps[:])
                return qk_g

            def emit_soft(t):
                """exp + softmax tail for tile t (sm(t) was made last iter)."""
                v_sb, sm = saved[t]
                ex = sfxp.tile([128, 1024], BF16, tag="ex")
                nc.scalar.activation(ex[:], sm[:], EXP)
                # garbage slots exp(0)=1: rowsum has exactly +120 extra
                sums = sfxp.tile([128, 8], F32, tag="sums")
                nc.vector.tensor_reduce(
                    out=sums[:], in_=ex[:].rearrange("p (h c) -> p h c", h=8),
                    axis=AXX, op=ADD)
                sumsc = sfxp.tile([128, 8], F32, tag="sumsc")
                nc.vector.tensor_scalar_add(sumsc[:], sums[:], -120.0)
                rec32 = sfxp.tile([128, 8], F32, tag="rec32")
                nc.vector.reciprocal(rec32[:], sumsc[:])
                rec = sfxp.tile([128, 8], BF16, tag="rec")
                nc.vector.tensor_copy(out=rec[:], in_=rec32[:])
                # normalize + zero garbage in one pass per head block
                attnb = attp.tile([128, 1024], BF16, tag="attnb")
                for j in range(8):
                    jb = slice(j * 128, (j + 1) * 128)
                    nc.vector.scalar_tensor_tensor(
                        out=attnb[:, jb], in0=ex[:, jb], scalar=rec[:, j:j + 1],
                        in1=bmask[:], op0=MULT, op1=MULT)
                attnT = attp.tile([128, 1024], BF16, tag="attnT")
                nc.vector.transpose(attnT[:], attnb[:])
                saved[t] = (v_sb, attnT)

            for g in range(nt // GSZ):
                qk_g = emit_group_qkT(g)
                for s in range(GSZ):
                    t = g * GSZ + s
                    tb = t - GSZ  # back-stage tile (softmax long done)
                    # ---- back stage part 1: AV matmuls
                    avt_ps = emit_back(tb) if (tb >= 0 and STAGE >= 5) else None
                    # ---- v natural [128 rows, 512]
                    v_ps = ps_v.tile([128, 512], F32, tag="v_ps")
                    for c in range(4):
                        nc.tensor.matmul(
                            v_ps[:],
                            lhsT=xfT[:, c * ROWS + t * 128: c * ROWS + (t + 1) * 128],
                            rhs=wq_sb[:, c * 1536 + 1024: c * 1536 + 1536],
                            start=(c == 0), stop=(c == 3),
                        )
                    # ---- back stage part 2: avT evac, out-proj, dma
                    if tb >= 0 and STAGE >= 5:
                        emit_back2(tb, avt_ps)
                    v_sb = vsbp.tile([128, 512], BF16, tag="v_sb")
                    nc.scalar.copy(out=v_sb[:], in_=v_ps[:])
                    if STAGE < 5:
                        nc.scalar.dma_start(out=out_d[t * RT:(t + 1) * RT, :],
                                            in_=v_sb[:])
                    if STAGE < 2:
                        continue

                    # ---- scores: even heads -> scA, odd -> scB
                    scA = ps_sc.tile([128, 512], F32, tag="scA")
                    scB = ps_sc.tile([128, 512], F32, tag="scB")
                    for j, h in enumerate(HORD):
                        pb = 64 * (h % 2)
                        qof = (h // 2) * 512 + s * 128
                        kof = (4 + h // 2) * 512 + s * 128
                        dst = scA if j < 4 else scB
                        nc.tensor.matmul(
                            dst[:, (j % 4) * 128:(j % 4) * 128 + 128],
                            lhsT=qk_g[pb:pb + 64, qof:qof + 128],
                            rhs=qk_g[pb:pb + 64, kof:kof + 128],
                            start=True, stop=True,
                        )
                    if STAGE < 3:
                        # still consume scA/scB so psum banks recycle
                        dbg = sfxp.tile([128, 1024], BF16, tag="sm")
                        nc.vector.tensor_copy(out=dbg[:, :512], in_=scA[:])
                        nc.vector.tensor_copy(out=dbg[:, 512:], in_=scB[:])
                        continue
                    # ---- mask-mult straight out of psum
                    mkp_b = mfull[:, t * 128:(t + 1) * 128]\
                        .rearrange("p (o c) -> p o c", o=1)\
                        .to_broadcast([128, 4, 128])
                    sm = sfxp.tile([128, 1024], BF16, tag="sm")
                    nc.vector.tensor_tensor(
                        out=sm[:, :512].rearrange("p (h c) -> p h c", h=4),
                        in0=scA[:].rearrange("p (h c) -> p h c", h=4),
                        in1=mkp_b, op=MULT)
                    nc.vector.tensor_tensor(
                        out=sm[:, 512:].rearrange("p (h c) -> p h c", h=4),
                        in0=scB[:].rearrange("p (h c) -> p h c", h=4),
                        in1=mkp_b, op=MULT)
                    saved[t] = (v_sb, sm)
                    if STAGE >= 4:
                        emit_soft(t)

            # drain last group's back stages
            if STAGE >= 5:
                for t in range(nt - GSZ, nt):
                    avt_ps = emit_back(t)
                    emit_back2(t, avt_ps)
    return nc


def host_inputs(x, mask, w_qkv, w_out):
    """Build per-core input maps (host-side layout/dtype prep only)."""
    bf = ml_dtypes.bfloat16
    x_rows = np.ascontiguousarray(x.reshape(PIX_TOTAL * NTOK, DIM)).astype(bf)
    wq = np.ascontiguousarray(w_qkv).astype(bf)
    wo = np.ascontiguousarray(w_out).astype(bf)

    # block indicator bmask[8*px + i, 8*px2 + j] = (px == px2), px over 16
    bm = np.zeros((16, 8, 16, 8), np.float32)
    for p in range(16):
        bm[p, :, p, :] = 1.0
    bmask = bm.reshape(128, 128).astype(bf)

    # mask' per row (px,i): cols (px'', j) = (px''==px%16) ? mask[px,i,j]*SCALE : 0
    m = mask.reshape(PIX_TOTAL, NTOK, NTOK).astype(np.float32)
    in_maps = []
    for cidx in range(N_CORES):
        xc = x_rows[cidx * ROWS:(cidx + 1) * ROWS]  # [8192, 512]
        xT = np.ascontiguousarray(xc.T)             # [512, 8192]
        mc = m[cidx * PIX_CORE:(cidx + 1) * PIX_CORE]  # [1024, 8, 8]
        mp = np.zeros((PIX_CORE, NTOK, 16, NTOK), np.float32)
        pl = np.arange(PIX_CORE) % 16
        mp[np.arange(PIX_CORE), :, pl, :] = mc * SCALE
        maskp = mp.reshape(ROWS, 128).astype(bf)
        in_maps.append({
            "xT": xT,
            "wqkv": wq,
            "wout": wo,
            "maskp": maskp,
            "bmask": bmask,
        })
    return in_maps


def kernel(x, mask, w_qkv, w_out, b_out):
    if "nc" not in _cache:
        nc0 = build_nc(NT)
        nc0.finalize()
        _cache["nc"] = nc0
    nc = _cache["nc"]
    in_maps = host_inputs(x, mask, w_qkv, w_out)
    trace = bool(os.environ.get("KERNEL_TRACE"))
    res = run_bass_kernel_spmd(nc, in_maps, list(range(N_CORES)), trace=trace)
    _cache["last_res"] = res
    outs = [np.asarray(r["out"]).astype(np.float32) for r in res.results]
    full = np.concatenate(outs, axis=0)  # [65536, 512]
    out = full.reshape(B, H, W, NTOK, DIM) + np.asarray(b_out, dtype=np.float32)
    return out.astype(np.float32)
